# revision 4
# baseline (speedup 1.0000x reference)
"""Causal self-attention (GPT-style, B=2 T=2048 C=1024 H=16) on 8 Trainium2 cores.

Sharding (Megatron-style, per spec hint): data-parallel over batch (cores 0-3
own b=0, cores 4-7 own b=1) x tensor-parallel over heads (4 heads/core,
c_attn column-split / c_proj row-split). Each core emits a partial [T, C]
output; the host unshard step sums the 4 partials per batch (the TP
all-reduce) and stacks the batches.

Per-core device program (all matmuls fp32 unless DT says otherwise):
  1. QKV projection from pre-transposed x^T (host provides x[b].T so the
     C-contraction lands on SBUF partitions without an on-chip transpose).
     Q,K are produced transposed ([head-dim, T] layout) for QK^T; V natural
     ([T, head-dim]) with a fused ones-column for softmax denominators.
  2. Causal flash-style attention per (head, 512-wide query slice): S^T
     blocks via PE, exp via ACT (scale=1/8 folded in), causal mask multiply
     on diagonal blocks, P^T@[V|1] accumulation -> unnormalized O^T plus
     denominator row; normalize via DMA-broadcast reciprocal.
  3. Output projection from O^T stacked [256, T] against W_proj rows.
"""

import numpy as np

import concourse.bass as bass  # noqa: F401  (re-exported types)
import concourse.mybir as mybir
import concourse.tile as tile
from concourse import bacc
from concourse.bass_utils import run_bass_kernel_spmd

B, T, C = 2, 2048, 1024
H, DH = 16, 64
NCORES = 8
GROUP = 4            # cores per batch (tensor-parallel group)
HPC = H // GROUP     # heads per core
P = 128
KO = C // P          # k-subtiles in the C contraction
TQ = 512             # query-slice width (max fp32 matmul free dim / PSUM bank)
NTS = T // TQ
NTK = T // P

F32 = mybir.dt.float32
BF16 = mybir.dt.bfloat16

_CACHE: dict = {}


def _build_nc(dt):
    nc = bacc.Bacc("TRN2", target_bir_lowering=False, debug=False, num_devices=NCORES)
    xT = nc.dram_tensor("xT", [C, T], dt, kind="ExternalInput")
    wqk = nc.dram_tensor("wqk", [C, 512], dt, kind="ExternalInput")
    wv = nc.dram_tensor("wv", [C, 256], dt, kind="ExternalInput")
    wp = nc.dram_tensor("wp", [2 * P, C], dt, kind="ExternalInput")
    bqk = nc.dram_tensor("bqk", [512], F32, kind="ExternalInput")
    bv = nc.dram_tensor("bv", [256], F32, kind="ExternalInput")
    bp = nc.dram_tensor("bp", [C], F32, kind="ExternalInput")
    mask = nc.dram_tensor("mask", [P, 896], dt, kind="ExternalInput")
    y = nc.dram_tensor("y", [T, C], F32, kind="ExternalOutput")

    with tile.TileContext(nc) as tc:
        _emit(tc, dt, xT, wqk, wv, wp, bqk, bv, bp, mask, y)
    nc.compile()
    return nc


def _emit(tc, dt, xT, wqk, wv, wp, bqk, bv, bp, mask, y):
    nc = tc.nc
    Exp = mybir.ActivationFunctionType.Exp
    Copy = mybir.ActivationFunctionType.Copy
    Ident = mybir.ActivationFunctionType.Identity

    with (
        tc.tile_pool(name="consts", bufs=1) as consts,
        tc.tile_pool(name="xp", bufs=2) as xp,
        tc.tile_pool(name="ptp", bufs=6) as ptp,
        tc.tile_pool(name="smp", bufs=3) as smp,
        tc.tile_pool(name="obp", bufs=3) as obp,
        tc.tile_pool(name="psp", bufs=4, space="PSUM") as psp,
        tc.tile_pool(name="psop", bufs=2, space="PSUM") as psop,
    ):
        # ---------------- constants / persistent tensors ----------------
        wqk_sb = consts.tile([P, KO, 512], dt, tag="wqk")
        nc.sync.dma_start(wqk_sb[:], wqk.ap().rearrange("(ko p) m -> p ko m", p=P))
        wv_sb = consts.tile([P, KO, 256], dt, tag="wv")
        nc.sync.dma_start(wv_sb[:], wv.ap().rearrange("(ko p) m -> p ko m", p=P))
        wp_sb = consts.tile([P, 2, C], dt, tag="wp")
        nc.sync.dma_start(wp_sb[:], wp.ap().rearrange("(g p) c -> p g c", p=P))
        bqk_sb = consts.tile([P, 4], F32, tag="bqk")
        nc.sync.dma_start(bqk_sb[:], bqk.ap().rearrange("(m p) -> p m", p=P))
        bv_sb = consts.tile([P, 256], F32, tag="bv")
        nc.sync.dma_start(bv_sb[:], bv.ap().partition_broadcast(P))
        bp_sb = consts.tile([P, C], F32, tag="bp")
        nc.sync.dma_start(bp_sb[:], bp.ap().partition_broadcast(P))
        mask_sb = consts.tile([P, 896], dt, tag="mask")
        nc.sync.dma_start(mask_sb[:], mask[:])

        QT = consts.tile([P, 2, T], dt, tag="QT")   # q^T, head h at (g=h//2, part 64*(h%2))
        KTt = consts.tile([P, 2, T], dt, tag="KT")  # k^T, same layout
        Vt = consts.tile([P, NTK, HPC, DH + 1], dt, tag="V")  # v natural + ones col
        Ot = consts.tile([P, 2, T], dt, tag="O")    # unnormalized-then-normalized O^T

        nc.any.memset(Vt[:, :, :, DH:DH + 1], 1.0)

        xTr = xT.ap().rearrange("(ko p) t -> p ko t", p=P)

        # ---------------- phase 1: QKV projections ----------------
        for ts in range(NTS):
            sl = slice(ts * TQ, (ts + 1) * TQ)
            xt = xp.tile([P, KO, TQ], dt, tag="xt")
            nc.sync.dma_start(xt[:], xTr[:, :, sl])
            # Q^T / K^T: out rows = 128-chunk of [q(h0,h1)|q(h2,h3)|k(h0,h1)|k(h2,h3)]
            for m in range(4):
                pm = psp.tile([P, TQ], F32, tag="mm")
                for k in range(KO):
                    nc.tensor.matmul(
                        pm[:], wqk_sb[:, k, m * P:(m + 1) * P], xt[:, k, :],
                        start=(k == 0), stop=(k == KO - 1),
                    )
                dest = QT if m < 2 else KTt
                nc.scalar.activation(dest[:, m % 2, sl], pm[:], Ident,
                                     bias=bqk_sb[:, m:m + 1])
            # V natural: 128-token chunks
            for j in range(TQ // P):
                tchunk = ts * (TQ // P) + j
                pv = psp.tile([P, TQ], F32, tag="mm")
                for k in range(KO):
                    nc.tensor.matmul(
                        pv[:, 0:256], xt[:, k, j * P:(j + 1) * P], wv_sb[:, k, :],
                        start=(k == 0), stop=(k == KO - 1),
                    )
                nc.vector.tensor_add(
                    Vt[:, tchunk, :, 0:DH],
                    pv[:, 0:256].rearrange("p (h d) -> p h d", h=HPC),
                    bv_sb[:].rearrange("p (h d) -> p h d", h=HPC),
                )

        # ---------------- phase 2: causal attention ----------------
        LA = 3  # lookahead distance between S^T production and P^T@V consumption
        for qs in range(NTS):
            qsl = slice(qs * TQ, (qs + 1) * TQ)
            ntk = 4 * qs + 4  # causal: tk tiles 0 .. 4qs+3
            for h in range(HPC):
                g, pb = h // 2, (h % 2) * DH
                po = psop.tile([DH + 1, TQ], F32, tag="pv")
                pts = []

                def emit_pv(tk, po=po, pts=pts, ntk=ntk, h=h):
                    nc.tensor.matmul(po[:], Vt[:, tk, h, :], pts[tk],
                                     start=(tk == 0), stop=(tk == ntk - 1))

                for tk in range(ntk):
                    pss = psp.tile([P, TQ], F32, tag="mm")
                    nc.tensor.matmul(
                        pss[:],
                        KTt[pb:pb + DH, g, tk * P:(tk + 1) * P],
                        QT[pb:pb + DH, g, qsl],
                        start=True, stop=True,
                    )
                    pt = ptp.tile([P, TQ], dt, tag="pt")
                    nc.scalar.activation(pt[:], pss[:], Exp, scale=0.125)
                    j = tk - 4 * qs
                    if j >= 0:  # diagonal band: apply causal mask
                        nc.vector.tensor_mul(
                            pt[:], pt[:], mask_sb[:, 384 - P * j:896 - P * j])
                    pts.append(pt[:])
                    if tk >= LA:
                        emit_pv(tk - LA)
                for tk in range(max(0, ntk - LA), ntk):
                    emit_pv(tk)

                # normalize: rows 0..63 = unnormalized O^T, row 64 = denominator
                den = smp.tile([1, TQ], F32, tag="den")
                nc.scalar.activation(den[:], po[DH:DH + 1, :], Copy)
                db = smp.tile([DH, TQ], F32, tag="db")
                nc.gpsimd.partition_broadcast(db[:], den[:])
                rec = smp.tile([DH, TQ], F32, tag="rec")
                nc.vector.reciprocal(rec[:], db[:])
                nc.vector.tensor_mul(Ot[pb:pb + DH, g, qsl], po[0:DH, :], rec[:])

        # ---------------- phase 3: output projection ----------------
        for t in range(T // P):
            for ns in range(C // TQ):
                pp = psp.tile([P, TQ], F32, tag="mm")
                for g in range(2):
                    nc.tensor.matmul(
                        pp[:], Ot[:, g, t * P:(t + 1) * P],
                        wp_sb[:, g, ns * TQ:(ns + 1) * TQ],
                        start=(g == 0), stop=(g == 1),
                    )
                ob = obp.tile([P, TQ], F32, tag="ob")
                nc.vector.tensor_add(ob[:], pp[:], bp_sb[:, ns * TQ:(ns + 1) * TQ])
                nc.sync.dma_start(y[t * P:(t + 1) * P, ns * TQ:(ns + 1) * TQ], ob[:])


def _np_dt(dt):
    if dt == BF16:
        import ml_dtypes
        return ml_dtypes.bfloat16
    return np.float32


def make_in_maps(x, W_attn, b_attn, W_proj, b_proj, dt=F32):
    """Shard full inputs into per-core input maps."""
    npdt = _np_dt(dt)
    x = np.asarray(x, np.float32)
    W_attn = np.asarray(W_attn, np.float32)
    b_attn = np.asarray(b_attn, np.float32)
    W_proj = np.asarray(W_proj, np.float32)
    b_proj = np.asarray(b_proj, np.float32)

    mask = (np.arange(896)[None, :] >= (np.arange(P)[:, None] + 384)).astype(npdt)
    in_maps = []
    for c in range(NCORES):
        b, hb = c // GROUP, c % GROUP
        cs = slice(hb * 256, (hb + 1) * 256)
        wq = W_attn[:, 0 * C:1 * C][:, cs]
        wk = W_attn[:, 1 * C:2 * C][:, cs]
        wv = W_attn[:, 2 * C:3 * C][:, cs]
        in_maps.append({
            "xT": np.ascontiguousarray(x[b].T).astype(npdt),
            "wqk": np.ascontiguousarray(np.concatenate([wq, wk], axis=1)).astype(npdt),
            "wv": np.ascontiguousarray(wv).astype(npdt),
            "wp": np.ascontiguousarray(W_proj[cs, :]).astype(npdt),
            "bqk": np.concatenate([b_attn[0 * C:1 * C][cs], b_attn[1 * C:2 * C][cs]]),
            "bv": np.ascontiguousarray(b_attn[2 * C:3 * C][cs]),
            "bp": (b_proj if hb == 0 else np.zeros_like(b_proj)),
            "mask": mask,
        })
    return in_maps


def get_nc(dt=F32):
    key = ("nc", str(dt))
    if key not in _CACHE:
        _CACHE[key] = _build_nc(dt)
    return _CACHE[key]


def unshard(results):
    y = np.zeros((B, T, C), np.float32)
    for c in range(NCORES):
        y[c // GROUP] += results[c]["y"]
    return y


def kernel(x, W_attn, b_attn, W_proj, b_proj):
    dt = F32
    nc = get_nc(dt)
    in_maps = make_in_maps(x, W_attn, b_attn, W_proj, b_proj, dt)
    res = run_bass_kernel_spmd(nc, in_maps, list(range(NCORES)))
    return unshard(res.results)


# revision 28
# speedup vs baseline: 18870.4134x; 18870.4134x over previous
"""Causal self-attention (GPT-style, B=2 T=2048 C=1024 H=16) on 8 Trainium2 cores.

Sharding (Megatron-style, per spec hint): data-parallel over batch (cores 0-3
own b=0, cores 4-7 own b=1) x tensor-parallel over heads (4 heads/core,
c_attn column-split / c_proj row-split). Each core emits a partial [T, C]
output; the host unshard step sums the 4 partials per batch (the TP
all-reduce) and stacks the batches.

Per-core device program (float32r matmuls — fp32 storage, relaxed-precision
PE mode at 4x the fp32 matmul rate; set ATTN_KERNEL_DT=f32 for full fp32):
  1. QKV projection from pre-transposed x^T (host provides x[b].T so the
     C-contraction lands on SBUF partitions without an on-chip transpose).
     Q,K are produced transposed ([head-dim, T] layout) for QK^T; V natural
     ([T, head-dim]) with a fused ones-column for softmax denominators.
  2. Causal flash-style attention per (head, 512-wide query slice): S^T
     blocks via PE, exp via ACT (scale=1/8 folded in), causal mask multiply
     on diagonal blocks, P^T@[V|1] accumulation -> unnormalized O^T plus
     denominator row; normalize via DMA-broadcast reciprocal.
  3. Output projection from O^T stacked [256, T] against W_proj rows.
"""

import numpy as np

import concourse.bass as bass  # noqa: F401  (re-exported types)
import concourse.mybir as mybir
import concourse.tile as tile
from concourse import bacc
from concourse.bass_utils import run_bass_kernel_spmd

B, T, C = 2, 2048, 1024
H, DH = 16, 64
NCORES = 8
GROUP = 4            # cores per batch (tensor-parallel group)
HPC = H // GROUP     # heads per core
P = 128
KO = C // P          # k-subtiles in the C contraction
TQ = 512             # query-slice width (max fp32 matmul free dim / PSUM bank)
NTS = T // TQ
NTK = T // P

F32 = mybir.dt.float32
BF16 = mybir.dt.bfloat16

_CACHE: dict = {}


def _build_nc(dt, n_iters=1, mm_r=False, phases=(1, 2, 3)):
    nc = bacc.Bacc("TRN2", target_bir_lowering=False, debug=False, num_devices=NCORES)
    xT = nc.dram_tensor("xT", [C, T], dt, kind="ExternalInput")
    wqk = nc.dram_tensor("wqk", [C, 512], dt, kind="ExternalInput")
    wv = nc.dram_tensor("wv", [C, 256], dt, kind="ExternalInput")
    wp = nc.dram_tensor("wp", [2 * P, C], dt, kind="ExternalInput")
    bqk = nc.dram_tensor("bqk", [512], F32, kind="ExternalInput")
    bv = nc.dram_tensor("bv", [256], F32, kind="ExternalInput")
    bp = nc.dram_tensor("bp", [C], F32, kind="ExternalInput")
    mask = nc.dram_tensor("mask", [P, 896], dt, kind="ExternalInput")
    y = nc.dram_tensor("y", [T, C], F32, kind="ExternalOutput")

    with tile.TileContext(nc) as tc:
        if n_iters == 1:
            _emit(tc, dt, xT, wqk, wv, wp, bqk, bv, bp, mask, y, mm_r, phases)
        else:
            with tc.For_i(0, n_iters, 1):
                _emit(tc, dt, xT, wqk, wv, wp, bqk, bv, bp, mask, y, mm_r, phases)
    nc.compile()
    return nc


def _emit(tc, dt, xT, wqk, wv, wp, bqk, bv, bp, mask, y, mm_r=False,
          phases=(1, 2, 3)):
    nc = tc.nc
    Exp = mybir.ActivationFunctionType.Exp
    Ident = mybir.ActivationFunctionType.Identity

    F32R = mybir.dt.float32r

    def mm(out, lhsT, rhs, **kw):
        if mm_r and lhsT.dtype == F32:
            lhsT, rhs = lhsT.bitcast(F32R), rhs.bitcast(F32R)
        nc.tensor.matmul(out, lhsT, rhs, **kw)

    with (
        tc.tile_pool(name="consts", bufs=1) as consts,
        tc.tile_pool(name="xp", bufs=2) as xp,
        tc.tile_pool(name="ptp", bufs=4) as ptp,
        tc.tile_pool(name="smp", bufs=3) as smp,
        tc.tile_pool(name="obp", bufs=3) as obp,
        tc.tile_pool(name="psp", bufs=2, space="PSUM") as psp,
        tc.tile_pool(name="psop", bufs=4, space="PSUM") as psop,
    ):
        # -------- constants (ordered so phase-1-critical loads come first) ----
        # wqk split per k-subtile so the first QK matmuls start early
        wqk_r = wqk.ap().rearrange("(ko p) m -> p ko m", p=P)
        wqk_sb = consts.tile([P, KO, 512], dt, tag="wqk")
        for k in range(KO):
            nc.sync.dma_start(wqk_sb[:, k], wqk_r[:, k])
        bqk_sb = consts.tile([P, 4], F32, tag="bqk")
        nc.sync.dma_start(bqk_sb[:], bqk.ap().rearrange("(m p) -> p m", p=P))
        wv_sb = consts.tile([P, KO, 256], dt, tag="wv")
        nc.sync.dma_start(wv_sb[:], wv.ap().rearrange("(ko p) m -> p ko m", p=P))
        bv_sb = consts.tile([P, 256], F32, tag="bv")
        nc.sync.dma_start(bv_sb[:], bv.ap().partition_broadcast(P))
        mask_sb = consts.tile([P, 896], dt, tag="mask")
        nc.sync.dma_start(mask_sb[:], mask[:])

        QT = consts.tile([P, 2, T], dt, tag="QT")   # q^T, head h at (g=h//2, part 64*(h%2))
        KTt = consts.tile([P, 2, T], dt, tag="KT")  # k^T, same layout
        Vt = consts.tile([P, NTK, HPC, DH + 1], dt, tag="V")  # v natural + ones col
        Ot = consts.tile([P, 2, T], dt, tag="O")    # unnormalized-then-normalized O^T

        # ones column for the softmax-denominator trick; mask col 895 is all-ones
        # (memset can't produce float32r, so copy ones from the mask instead)
        nc.vector.tensor_copy(
            Vt[:, :, :, DH:DH + 1],
            mask_sb[:, 895:896].to_broadcast((P, NTK, HPC, 1)),
        )

        xTr = xT.ap().rearrange("(ko p) t -> p ko t", p=P)

        # ---------------- phase 1: QKV projections ----------------
        for ts in range(NTS if 1 in phases else 0):
            sl = slice(ts * TQ, (ts + 1) * TQ)
            xt = xp.tile([P, KO, TQ], dt, tag="xt")
            for k in range(KO):
                nc.sync.dma_start(xt[:, k], xTr[:, k, sl])
            # Q^T / K^T: out rows = 128-chunk of [q(h0,h1)|q(h2,h3)|k(h0,h1)|k(h2,h3)]
            for m in range(4):
                pm = psp.tile([P, 2 * TQ], F32, tag="mm")
                for k in range(KO):
                    mm(
                        pm[:, 0:TQ], wqk_sb[:, k, m * P:(m + 1) * P], xt[:, k, :],
                        start=(k == 0), stop=(k == KO - 1),
                    )
                dest = QT if m < 2 else KTt
                nc.scalar.activation(dest[:, m % 2, sl], pm[:, 0:TQ], Ident,
                                     bias=bqk_sb[:, m:m + 1])
            # V natural: 128-token chunks
            for j in range(TQ // P):
                tchunk = ts * (TQ // P) + j
                pv = psp.tile([P, 2 * TQ], F32, tag="mm")
                for k in range(KO):
                    mm(
                        pv[:, 0:256], xt[:, k, j * P:(j + 1) * P], wv_sb[:, k, :],
                        start=(k == 0), stop=(k == KO - 1),
                    )
                nc.vector.tensor_add(
                    Vt[:, tchunk, :, 0:DH],
                    pv[:, 0:256].rearrange("p (h d) -> p h d", h=HPC),
                    bv_sb[:].rearrange("p (h d) -> p h d", h=HPC),
                )

        # late consts: only needed by the (interleaved) output projection
        wp_sb = consts.tile([P, 2, C], dt, tag="wp")
        nc.sync.dma_start(wp_sb[:], wp.ap().rearrange("(g p) c -> p g c", p=P))
        bp_sb = consts.tile([P, C], F32, tag="bp")
        nc.sync.dma_start(bp_sb[:], bp.ap().partition_broadcast(P))

        def emit_proj(t):
            # y[t*128:(t+1)*128, :] = O^T[:, t-chunk].T @ Wp + bp
            pp = psp.tile([P, 2 * TQ], F32, tag="mm")
            for ns in range(C // TQ):
                for g in range(2):
                    mm(
                        pp[:, ns * TQ:(ns + 1) * TQ], Ot[:, g, t * P:(t + 1) * P],
                        wp_sb[:, g, ns * TQ:(ns + 1) * TQ],
                        start=(g == 0), stop=(g == 1),
                    )
            ob = obp.tile([P, C], F32, tag="ob")
            nc.vector.tensor_add(ob[:], pp[:], bp_sb[:])
            nc.sync.dma_start(y[t * P:(t + 1) * P, :], ob[:])

        # ------- phase 2: causal attention (+ interleaved output proj) -------
        LA = 2  # lookahead in tk-PAIRS between S^T/exp production and P^T@V

        def emit_norm(job):
            # normalize: rows 0..63 = unnormalized O^T, row 64 = denominator.
            # Runs lagged (one head behind) so its latency hides under the
            # next head's S^T/PV stream.
            po, g, pb, qsl = job
            den = smp.tile([1, TQ], F32, tag="den")
            nc.vector.tensor_copy(den[:], po[DH:DH + 1, :])
            db = smp.tile([DH, TQ], F32, tag="db")
            nc.gpsimd.partition_broadcast(db[:], den[:])
            rec = smp.tile([DH, TQ], F32, tag="rec")
            nc.vector.reciprocal_approx_fast(rec[:], db[:])
            nc.vector.tensor_mul(Ot[pb:pb + DH, g, qsl], po[0:DH, :], rec[:])

        pending_norm = []
        for qs in range(NTS if 2 in phases else 0):
            qsl = slice(qs * TQ, (qs + 1) * TQ)
            ntk = 4 * qs + 4  # causal: tk tiles 0 .. 4qs+3
            for hp in range(HPC // 2):
                # head pair (a, b) = (2hp, 2hp+1): a on partitions 0-63,
                # b on 64-127 -> the two S^T matmuls use disjoint PE row
                # groups and run concurrently (2x row tiling)
                g = hp
                ha, hb = 2 * hp, 2 * hp + 1
                po_a = psop.tile([DH + 1, TQ], F32, tag="pv")
                po_b = psop.tile([DH + 1, TQ], F32, tag="pv")
                pts = []

                def emit_pv(tk, po_a=po_a, po_b=po_b, pts=pts, ntk=ntk,
                            ha=ha, hb=hb):
                    pt2 = pts[tk]
                    mm(po_a[:], Vt[:, tk, ha, :], pt2[:, 0:TQ],
                       start=(tk == 0), stop=(tk == ntk - 1))
                    mm(po_b[:], Vt[:, tk, hb, :], pt2[:, TQ:2 * TQ],
                       start=(tk == 0), stop=(tk == ntk - 1))

                for tk in range(ntk):
                    # S^T for both heads of the pair at this tk: concurrent
                    # row-tiled matmuls into one [128, 1024] psum -> one exp
                    pss = psp.tile([P, 2 * TQ], F32, tag="mm")
                    mm(
                        pss[:, 0:TQ],
                        KTt[0:DH, g, tk * P:(tk + 1) * P],
                        QT[0:DH, g, qsl],
                        start=True, stop=True,
                    )
                    mm(
                        pss[:, TQ:2 * TQ],
                        KTt[DH:P, g, tk * P:(tk + 1) * P],
                        QT[DH:P, g, qsl],
                        start=True, stop=True,
                    )
                    pt2 = ptp.tile([P, 2 * TQ], dt, tag="pt")
                    nc.scalar.activation(pt2[:], pss[:], Exp, scale=0.125)
                    j = tk - 4 * qs
                    if j >= 0:  # diagonal band: mask only the affected cols
                        w = min(TQ, P * (j + 1))
                        msl = mask_sb[:, 384 - P * j:384 - P * j + w]
                        nc.vector.tensor_mul(pt2[:, 0:w], pt2[:, 0:w], msl)
                        nc.vector.tensor_mul(pt2[:, TQ:TQ + w],
                                             pt2[:, TQ:TQ + w], msl)
                    pts.append(pt2[:])
                    if tk == 0:
                        while pending_norm:
                            emit_norm(pending_norm.pop(0))
                    if tk >= LA:
                        emit_pv(tk - LA)
                for tk in range(max(0, ntk - LA), ntk):
                    emit_pv(tk)
                pending_norm.append((po_a, g, 0, qsl))
                pending_norm.append((po_b, g, DH, qsl))

            while pending_norm:  # proj below needs every head of qs normalized
                emit_norm(pending_norm.pop(0))

            if 3 in phases:  # project this query slice's finished rows
                for t in range(4 * qs, 4 * qs + 4):
                    emit_proj(t)

        if 3 in phases and 2 not in phases:
            for t in range(T // P):
                emit_proj(t)


def _np_dt(dt):
    if dt == BF16:
        import ml_dtypes
        return ml_dtypes.bfloat16
    return np.float32


def make_in_maps(x, W_attn, b_attn, W_proj, b_proj, dt=F32):
    """Shard full inputs into per-core input maps."""
    npdt = _np_dt(dt)
    x = np.asarray(x, np.float32)
    W_attn = np.asarray(W_attn, np.float32)
    b_attn = np.asarray(b_attn, np.float32)
    W_proj = np.asarray(W_proj, np.float32)
    b_proj = np.asarray(b_proj, np.float32)

    mask = (np.arange(896)[None, :] >= (np.arange(P)[:, None] + 384)).astype(npdt)
    in_maps = []
    for c in range(NCORES):
        b, hb = c // GROUP, c % GROUP
        cs = slice(hb * 256, (hb + 1) * 256)
        wq = W_attn[:, 0 * C:1 * C][:, cs]
        wk = W_attn[:, 1 * C:2 * C][:, cs]
        wv = W_attn[:, 2 * C:3 * C][:, cs]
        in_maps.append({
            "xT": np.ascontiguousarray(x[b].T).astype(npdt),
            "wqk": np.ascontiguousarray(np.concatenate([wq, wk], axis=1)).astype(npdt),
            "wv": np.ascontiguousarray(wv).astype(npdt),
            "wp": np.ascontiguousarray(W_proj[cs, :]).astype(npdt),
            "bqk": np.concatenate([b_attn[0 * C:1 * C][cs], b_attn[1 * C:2 * C][cs]]),
            "bv": np.ascontiguousarray(b_attn[2 * C:3 * C][cs]),
            "bp": (b_proj if hb == 0 else np.zeros_like(b_proj)),
            "mask": mask,
        })
    return in_maps


def get_nc(dt=F32, n_iters=1, mm_r=False, phases=(1, 2, 3)):
    key = ("nc", str(dt), n_iters, mm_r, tuple(phases))
    if key not in _CACHE:
        _CACHE[key] = _build_nc(dt, n_iters, mm_r, phases)
    return _CACHE[key]


def unshard(results):
    y = np.zeros((B, T, C), np.float32)
    for c in range(NCORES):
        y[c // GROUP] += results[c]["y"]
    return y


def kernel(x, W_attn, b_attn, W_proj, b_proj):
    import os
    dt = F32 if os.environ.get("ATTN_KERNEL_DT") == "f32" else mybir.dt.float32r
    nc = get_nc(dt)
    in_maps = make_in_maps(x, W_attn, b_attn, W_proj, b_proj, dt)
    res = run_bass_kernel_spmd(nc, in_maps, list(range(NCORES)))
    return unshard(res.results)
